# revision 20
# baseline (speedup 1.0000x reference)
"""Trainium2 Bass kernel for the CRA relation module (bf16 v2).

Math (per sample, derived from symmetry of A = cat_phi cat_phi^T):
    phi_x = relu(x@W1p + b1p), phi_y = relu(y@W2p + b2p)
    s  = u3 + phi_x^T z[:196] + phi_y^T z[196:]      (768-vector)
    W  = [phi_x|phi_y] @ s + c0                       (392 scalars)
    out = x * W[:196] + y * W[196:]

Device schedule per group of G=2 samples (8 groups/core, 16 samples/core,
pure data-parallel over 8 cores):
  PE   : 72 bf16 matmuls (768x768 "1x1 conv", both streams), then per
         sample 6 matvec matmuls with lhsT = s' replicated to 128 cols
         -> psw[128,392] (W on every partition)
  ACT  : 12 relu evictions PSUM->SBUF bf16 (strided out AP fills both
         samples' phi slots), 2 wxy evictions (+c0 -> bf16)
  DVE  : 12 z-weighted reductions (scalar_tensor_tensor accum), s'
         replication (tensor_scalar with per-partition t), x*Wx mult,
         final add
  Pool : y*Wy mult (tensor_tensor, broadcast-d AP on Wy)
  DMA  : xg/yg in (bf16), out (bf16; host casts fp32)

Layouts: feature-major. xg[group] = [128, 6*392] bf16, each 392 block =
[x_a(196) | x_b(196)] for one cin tile. Broadcast of W along the d-repeat
uses 0-stride APs (verified supported by walrus + sim).
"""

import numpy as np
import ml_dtypes
from contextlib import ExitStack

import concourse.bass as bass
import concourse.tile as tile
import concourse.mybir as mybir
from concourse.bass_utils import run_bass_kernel_spmd

F32 = mybir.dt.float32
BF16 = mybir.dt.bfloat16
ALU = mybir.AluOpType
ACTF = mybir.ActivationFunctionType

B, N, C = 128, 196, 768
NCORES = 8
S = B // NCORES          # 16 samples per core
G = 2                    # samples per group
NG = S // G              # 8 groups per core
DT = C // 128            # 6 feature tiles
W2T = 2 * N              # 392
XW = DT * W2T            # 2352 cols per xg tile
ON = DT * N              # 1176 out cols per sample


def build_bass(c0: float, split: bool = True) -> bass.Bass:
    nc = bass.Bass()
    xg_d = nc.declare_dram_parameter("xg", [NG, 128, XW], BF16, isOutput=False)
    yg_d = nc.declare_dram_parameter("yg", [NG, 128, XW], BF16, isOutput=False)
    w1_d = nc.declare_dram_parameter("w1", [C, C], BF16, isOutput=False)
    w2_d = nc.declare_dram_parameter("w2", [C, C], BF16, isOutput=False)
    zb_d = nc.declare_dram_parameter("zb", [128, W2T], BF16, isOutput=False)
    u3r_d = nc.declare_dram_parameter("u3r", [128, C], BF16, isOutput=False)
    b1_d = nc.declare_dram_parameter("b1", [128, DT], F32, isOutput=False)
    b2_d = nc.declare_dram_parameter("b2", [128, DT], F32, isOutput=False)
    out_d = nc.declare_dram_parameter("out", [S, 128, ON], BF16, isOutput=True)

    with tile.TileContext(nc) as tc, ExitStack() as ctx:
        const = ctx.enter_context(tc.tile_pool(name="const", bufs=1))

        # Weights + consts go on the ACT DMA queue so the first groups'
        # xg/yg (SP queue) land in parallel; k=0 weight tiles first.
        w1_sb, w2_sb = [], []
        for k in range(DT):
            t1 = const.tile([128, C], BF16, tag=f"w1_{k}")
            nc.scalar.dma_start(out=t1[:], in_=w1_d[k * 128:(k + 1) * 128, :])
            w1_sb.append(t1)
            t2 = const.tile([128, C], BF16, tag=f"w2_{k}")
            nc.scalar.dma_start(out=t2[:], in_=w2_d[k * 128:(k + 1) * 128, :])
            w2_sb.append(t2)

        zb = const.tile([128, W2T], BF16, tag="zb")
        nc.scalar.dma_start(out=zb[:], in_=zb_d[:, :])
        u3rep = const.tile([128, C], BF16, tag="u3r")
        nc.scalar.dma_start(out=u3rep[:], in_=u3r_d[:, :])
        b1t = const.tile([128, DT], F32, tag="b1")
        nc.scalar.dma_start(out=b1t[:], in_=b1_d[:, :])
        b2t = const.tile([128, DT], F32, tag="b2")
        nc.scalar.dma_start(out=b2t[:], in_=b2_d[:, :])
        ones_bf = const.tile([128, 2], BF16, tag="ones_bf")
        nc.vector.memset(ones_bf[:], 1.0)
        # Absorb bias-tile DMA deps into ACT program order once (ISA
        # Activation descriptor holds a single sync-wait).
        warm1 = const.tile([128, 1], F32, tag="warm1")
        warm2 = const.tile([128, 1], F32, tag="warm2")
        nc.scalar.activation(warm1[:], b1t[:, 0:1], ACTF.Copy)
        nc.scalar.activation(warm2[:], b2t[:, 0:1], ACTF.Copy)

        xin = ctx.enter_context(tc.tile_pool(name="xin", bufs=3))
        phip = ctx.enter_context(tc.tile_pool(name="phi", bufs=3))
        sp = ctx.enter_context(tc.tile_pool(name="sp", bufs=3))
        op = ctx.enter_context(tc.tile_pool(name="op", bufs=2))
        ps = ctx.enter_context(tc.tile_pool(name="ps", bufs=2, space="PSUM"))

        def strided2(tile_, offset, step, n, parts=128):
            """AP [parts, 2 (col step), n] at col offset within the tile."""
            t = tile_[:]
            return bass.AP(t.tensor, t.offset + offset,
                           [[t.ap[0][0], parts], [step, 2], [1, n]])

        def sample_ap(tile_, i):
            """AP [128, DT (step W2T), N]: sample i's stream cols of xg/yg."""
            t = tile_[:]
            return bass.AP(t.tensor, t.offset + i * N,
                           [[t.ap[0][0], 128], [W2T, DT], [1, N]])

        def bcast_ap(tile_, offset, n):
            """AP [128, DT (stride 0), n]: wxy cols broadcast over d."""
            t = tile_[:]
            return bass.AP(t.tensor, t.offset + offset,
                           [[t.ap[0][0], 128], [0, DT], [1, n]])

        def compact_ap(tile_, parts=128):
            t = tile_[:]
            return bass.AP(t.tensor, t.offset,
                           [[t.ap[0][0], parts], [N, DT], [1, N]])

        def emit_mains(g):
            xg = xin.tile([128, XW], BF16, tag="xg", name="xg")
            yg = xin.tile([128, XW], BF16, tag="yg", name="yg")
            nc.sync.dma_start(out=xg[:], in_=xg_d[g])
            nc.sync.dma_start(out=yg[:], in_=yg_d[g])
            # phi[d]: [128, 784] = [sample_a(phix|phiy) | sample_b(...)]
            phi = [phip.tile([128, 2 * W2T], BF16, tag=f"phi_{d}",
                             name=f"phi_{d}") for d in range(DT)]
            for d in range(DT):
                psx = ps.tile([128, W2T], F32, tag="psx", name="psx", bufs=3)
                psy = ps.tile([128, W2T], F32, tag="psy", name="psy", bufs=3)
                for k in range(DT):
                    nc.tensor.matmul(psx[:], w1_sb[k][:, d * 128:(d + 1) * 128],
                                     xg[:, k * W2T:(k + 1) * W2T],
                                     start=(k == 0), stop=(k == DT - 1))
                for k in range(DT):
                    nc.tensor.matmul(psy[:], w2_sb[k][:, d * 128:(d + 1) * 128],
                                     yg[:, k * W2T:(k + 1) * W2T],
                                     start=(k == 0), stop=(k == DT - 1))
                # relu evictions: psx = [a|b] of stream x -> phi[d] strided
                outx = strided2(phi[d], 0, W2T, N)
                outy = strided2(phi[d], N, W2T, N)
                nc.scalar.activation(outx, psx[:], ACTF.Relu, bias=b1t[:, d:d + 1])
                nc.scalar.activation(outy, psy[:], ACTF.Relu, bias=b2t[:, d:d + 1])
            return xg, yg, phi

        def emit_sred(g, phi):
            """DVE z-weighted reductions + s' build; overlaps mains(g)."""
            sreps = []
            for i in range(G):
                t_sb = sp.tile([128, DT], BF16, tag=f"t_{i}", name=f"t_{i}")
                s_rep = sp.tile([128, C], BF16, tag=f"srep_{i}",
                                name=f"srep_{i}")
                for d in range(DT):
                    scr = sp.tile([128, W2T], BF16, tag="scr", name="scr")
                    nc.vector.scalar_tensor_tensor(
                        out=scr[:], in0=phi[d][:, i * W2T:(i + 1) * W2T],
                        scalar=ones_bf[:, 0:1], in1=zb[:], op0=ALU.mult,
                        op1=ALU.mult, accum_out=t_sb[:, d:d + 1])
                # s_rep = u3 + t (t broadcast 128-wide per d-block), bf16
                tb = t_sb[:]
                t_bcast = bass.AP(tb.tensor, tb.offset,
                                  [[tb.ap[0][0], 128], [1, DT], [0, 128]])
                sr = s_rep[:]
                sr3 = bass.AP(sr.tensor, sr.offset,
                              [[sr.ap[0][0], 128], [128, DT], [1, 128]])
                u3r3 = u3rep[:].rearrange("p (d c) -> p d c", d=DT, c=128)
                nc.vector.tensor_tensor(sr3, u3r3, t_bcast, ALU.add)
                sreps.append(s_rep)
            return sreps

        def emit_rest(g, xg, yg, phi, sreps, last):
            """psw matvec + W eviction + final reweighting for group g."""
            for i in range(G):
                psw = ps.tile([128, W2T], F32, tag=f"psw_{i}",
                              name=f"psw_{i}", bufs=1)
                for d in range(DT):
                    nc.tensor.matmul(psw[:], sreps[i][:, d * 128:(d + 1) * 128],
                                     phi[d][:, i * W2T:(i + 1) * W2T],
                                     start=(d == 0), stop=(d == DT - 1))
                wxy = sp.tile([128, W2T], BF16, tag=f"wxy_{i}",
                              name=f"wxy_{i}")
                nc.scalar.activation(wxy[:], psw[:], ACTF.Copy, bias=c0)

                # out = x*Wx (DVE) ; y*Wy (Pool) ; add (Pool; DVE when last)
                gx = op.tile([128, ON], BF16, tag=f"gx_{i}", name=f"gx_{i}")
                gy = op.tile([128, ON], BF16, tag=f"gy_{i}", name=f"gy_{i}")
                nc.vector.tensor_tensor(compact_ap(gx), sample_ap(xg, i),
                                        bcast_ap(wxy, 0, N), ALU.mult)
                nc.vector.tensor_tensor(compact_ap(gy), sample_ap(yg, i),
                                        bcast_ap(wxy, N, N), ALU.mult)
                # final add happens in the DMA engine: out = gx, then += gy
                # (same SP queue -> ordered)
                nc.sync.dma_start(out=out_d[G * g + i], in_=gx[:])
                nc.gpsimd.dma_start(out=out_d[G * g + i], in_=gy[:],
                                    accum_op=ALU.add)

        prev = None
        for g in range(NG):
            cur = emit_mains(g)
            sreps = emit_sred(g, cur[2])
            if prev is not None:
                pg, (pxg, pyg, pphi), psreps = prev
                emit_rest(pg, pxg, pyg, pphi, psreps, False)
            prev = (g, cur, sreps)
        pg, (pxg, pyg, pphi), psreps = prev
        emit_rest(pg, pxg, pyg, pphi, psreps, True)

    if split:
        _split_multi_waits(nc)
    return nc


def _split_multi_waits(nc):
    """This walrus build accepts at most ONE sync-wait per TPB instruction;
    the Tile scheduler emits several. Hoist all but the last wait onto
    same-engine EventSemaphore ops placed immediately before."""
    import json
    data = json.loads(nc.to_json_bytes())
    n = 0
    for fn in data["functions"]:
        for blk in fn["blocks"]:
            out = []
            for inst in blk["instructions"]:
                si = inst.get("sync_info")
                ow = (si or {}).get("on_wait") or []
                if len(ow) > 1:
                    for w in ow[:-1]:
                        n += 1
                        out.append({
                            "name": f"eswait_{n}",
                            "opcode": "EventSemaphore",
                            "engine": inst["engine"],
                            "ins": [],
                            "outs": [],
                            "sync_info": {"on_wait": [w], "on_update": []},
                        })
                    si["on_wait"] = [ow[-1]]
                out.append(inst)
            blk["instructions"] = out
    nc.m = mybir.module_from_json_bytes(json.dumps(data).encode())
    return nc


def prep_host(inputs: dict):
    bf = ml_dtypes.bfloat16
    x = np.ascontiguousarray(np.asarray(inputs["x"], dtype=np.float32))
    y = np.ascontiguousarray(np.asarray(inputs["y"], dtype=np.float32))
    W1 = np.asarray(inputs["W1"], dtype=np.float32)
    W2 = np.asarray(inputs["W2"], dtype=np.float32)
    g1 = np.asarray(inputs["g1"], dtype=np.float32)
    g2 = np.asarray(inputs["g2"], dtype=np.float32)
    b1 = np.asarray(inputs["b1"], dtype=np.float32)
    b2 = np.asarray(inputs["b2"], dtype=np.float32)
    be1 = np.asarray(inputs["be1"], dtype=np.float32)
    be2 = np.asarray(inputs["be2"], dtype=np.float32)
    W3 = np.asarray(inputs["W3"], dtype=np.float32)
    b3 = np.asarray(inputs["b3"], dtype=np.float32)
    W4 = np.asarray(inputs["W4"], dtype=np.float32)
    b4 = np.asarray(inputs["b4"], dtype=np.float32)
    W5 = np.asarray(inputs["W5"], dtype=np.float32)
    b5 = np.asarray(inputs["b5"], dtype=np.float32)

    W1p = np.ascontiguousarray(W1 * g1[None, :]).astype(bf)
    W2p = np.ascontiguousarray(W2 * g2[None, :]).astype(bf)
    b1p = b1 * g1 + be1
    b2p = b2 * g2 + be2
    W5a, W5b = W5[:C, 0], W5[C:, 0]
    u3 = (W3 @ W5a).astype(np.float32)
    u4 = (W4 @ W5b).astype(np.float32)
    z = (u4[:2 * N] + u4[2 * N:]).astype(np.float32)
    c0 = float(b3 @ W5a + b4 @ W5b + b5[0])

    # [B,N,C] -> per-core groups [M, NG, 128, 6*392], blocks [x_a | x_b]
    def pack(a):
        at = a.transpose(0, 2, 1).reshape(NCORES, S, DT, 128, N)
        pair = at.reshape(NCORES, NG, G, DT, 128, N)
        gg = np.concatenate([pair[:, :, 0], pair[:, :, 1]], axis=-1)
        return np.ascontiguousarray(
            gg.transpose(0, 1, 3, 2, 4).reshape(NCORES, NG, 128, XW)).astype(bf)

    XG, YG = pack(x), pack(y)
    zbv = np.ascontiguousarray(np.broadcast_to(z[None, :], (128, W2T))).astype(bf)
    u3t = u3.reshape(DT, 128).T                                 # [128, 6]
    u3r = np.ascontiguousarray(np.repeat(u3t, 128, axis=1)).astype(bf)
    b1t = np.ascontiguousarray(b1p.reshape(DT, 128).T)
    b2t = np.ascontiguousarray(b2p.reshape(DT, 128).T)

    in_maps = []
    for cidx in range(NCORES):
        in_maps.append({
            "xg": XG[cidx], "yg": YG[cidx], "w1": W1p, "w2": W2p,
            "zb": zbv, "u3r": u3r, "b1": b1t, "b2": b2t,
        })
    return in_maps, c0, x, y


def unpack_out(results) -> np.ndarray:
    outs = []
    for cidx in range(NCORES):
        o = np.asarray(results[cidx]["out"]).astype(np.float32)  # [S,128,ON]
        o = o.reshape(S, 128, DT, N).transpose(0, 2, 1, 3).reshape(S, C, N)
        outs.append(o.transpose(0, 2, 1))     # [S, N, C]
    return np.ascontiguousarray(np.concatenate(outs, axis=0))


def kernel(**inputs) -> np.ndarray:
    in_maps, c0, _, _ = prep_host(inputs)
    nc = build_bass(c0)
    res = run_bass_kernel_spmd(nc, in_maps, list(range(NCORES)))
    return unpack_out(res.results)


# revision 22
# speedup vs baseline: 1.0422x; 1.0422x over previous
"""Trainium2 Bass kernel for the CRA relation module (bf16 v2).

Math (per sample, derived from symmetry of A = cat_phi cat_phi^T):
    phi_x = relu(x@W1p + b1p), phi_y = relu(y@W2p + b2p)
    s  = u3 + phi_x^T z[:196] + phi_y^T z[196:]      (768-vector)
    W  = [phi_x|phi_y] @ s + c0                       (392 scalars)
    out = x * W[:196] + y * W[196:]

Device schedule per group of G=2 samples (8 groups/core, 16 samples/core,
pure data-parallel over 8 cores):
  PE   : 72 bf16 matmuls (768x768 "1x1 conv", both streams), then per
         sample 6 matvec matmuls with lhsT = s' replicated to 128 cols
         -> psw[128,392] (W on every partition)
  ACT  : 12 relu evictions PSUM->SBUF bf16 (strided out AP fills both
         samples' phi slots), 2 wxy evictions (+c0 -> bf16)
  DVE  : 12 z-weighted reductions (scalar_tensor_tensor accum), s'
         replication (tensor_scalar with per-partition t), x*Wx mult,
         final add
  Pool : y*Wy mult (tensor_tensor, broadcast-d AP on Wy)
  DMA  : xg/yg in (bf16), out (bf16; host casts fp32)

Layouts: feature-major. xg[group] = [128, 6*392] bf16, each 392 block =
[x_a(196) | x_b(196)] for one cin tile. Broadcast of W along the d-repeat
uses 0-stride APs (verified supported by walrus + sim).
"""

import numpy as np
import ml_dtypes
from contextlib import ExitStack

import concourse.bass as bass
import concourse.tile as tile
import concourse.mybir as mybir
from concourse.bass_utils import run_bass_kernel_spmd

F32 = mybir.dt.float32
BF16 = mybir.dt.bfloat16
ALU = mybir.AluOpType
ACTF = mybir.ActivationFunctionType

B, N, C = 128, 196, 768
NCORES = 8
S = B // NCORES          # 16 samples per core
G = 2                    # samples per group
NG = S // G              # 8 groups per core
DT = C // 128            # 6 feature tiles
W2T = 2 * N              # 392
XW = DT * W2T            # 2352 cols per xg tile
ON = DT * N              # 1176 out cols per sample


def build_bass(c0: float, split: bool = True) -> bass.Bass:
    nc = bass.Bass()
    xg_d = nc.declare_dram_parameter("xg", [NG, 128, XW], BF16, isOutput=False)
    yg_d = nc.declare_dram_parameter("yg", [NG, 128, XW], BF16, isOutput=False)
    w1_d = nc.declare_dram_parameter("w1", [C, C], BF16, isOutput=False)
    w2_d = nc.declare_dram_parameter("w2", [C, C], BF16, isOutput=False)
    zb_d = nc.declare_dram_parameter("zb", [128, W2T], BF16, isOutput=False)
    u3r_d = nc.declare_dram_parameter("u3r", [128, C], BF16, isOutput=False)
    b1_d = nc.declare_dram_parameter("b1", [128, DT], F32, isOutput=False)
    b2_d = nc.declare_dram_parameter("b2", [128, DT], F32, isOutput=False)
    out_d = nc.declare_dram_parameter("out", [S, 128, ON], BF16, isOutput=True)

    with tile.TileContext(nc) as tc, ExitStack() as ctx:
        const = ctx.enter_context(tc.tile_pool(name="const", bufs=1))

        # Weights + consts go on the ACT DMA queue so the first groups'
        # xg/yg (SP queue) land in parallel; k=0 weight tiles first.
        w1_sb, w2_sb = [], []
        for k in range(DT):
            t1 = const.tile([128, C], BF16, tag=f"w1_{k}")
            nc.scalar.dma_start(out=t1[:], in_=w1_d[k * 128:(k + 1) * 128, :])
            w1_sb.append(t1)
            t2 = const.tile([128, C], BF16, tag=f"w2_{k}")
            nc.scalar.dma_start(out=t2[:], in_=w2_d[k * 128:(k + 1) * 128, :])
            w2_sb.append(t2)

        zb = const.tile([128, W2T], BF16, tag="zb")
        nc.scalar.dma_start(out=zb[:], in_=zb_d[:, :])
        u3rep = const.tile([128, C], BF16, tag="u3r")
        nc.scalar.dma_start(out=u3rep[:], in_=u3r_d[:, :])
        b1t = const.tile([128, DT], F32, tag="b1")
        nc.scalar.dma_start(out=b1t[:], in_=b1_d[:, :])
        b2t = const.tile([128, DT], F32, tag="b2")
        nc.scalar.dma_start(out=b2t[:], in_=b2_d[:, :])
        ones_bf = const.tile([128, 2], BF16, tag="ones_bf")
        nc.vector.memset(ones_bf[:], 1.0)
        # Absorb bias-tile DMA deps into ACT program order once (ISA
        # Activation descriptor holds a single sync-wait).
        warm1 = const.tile([128, 1], F32, tag="warm1")
        warm2 = const.tile([128, 1], F32, tag="warm2")
        nc.scalar.activation(warm1[:], b1t[:, 0:1], ACTF.Copy)
        nc.scalar.activation(warm2[:], b2t[:, 0:1], ACTF.Copy)

        xin = ctx.enter_context(tc.tile_pool(name="xin", bufs=3))
        phip = ctx.enter_context(tc.tile_pool(name="phi", bufs=3))
        sp = ctx.enter_context(tc.tile_pool(name="sp", bufs=3))
        op = ctx.enter_context(tc.tile_pool(name="op", bufs=2))
        ps = ctx.enter_context(tc.tile_pool(name="ps", bufs=2, space="PSUM"))

        def strided2(tile_, offset, step, n, parts=128):
            """AP [parts, 2 (col step), n] at col offset within the tile."""
            t = tile_[:]
            return bass.AP(t.tensor, t.offset + offset,
                           [[t.ap[0][0], parts], [step, 2], [1, n]])

        def sample_ap(tile_, i):
            """AP [128, DT (step W2T), N]: sample i's stream cols of xg/yg."""
            t = tile_[:]
            return bass.AP(t.tensor, t.offset + i * N,
                           [[t.ap[0][0], 128], [W2T, DT], [1, N]])

        def bcast_ap(tile_, offset, n):
            """AP [128, DT (stride 0), n]: wxy cols broadcast over d."""
            t = tile_[:]
            return bass.AP(t.tensor, t.offset + offset,
                           [[t.ap[0][0], 128], [0, DT], [1, n]])

        def compact_ap(tile_, parts=128):
            t = tile_[:]
            return bass.AP(t.tensor, t.offset,
                           [[t.ap[0][0], parts], [N, DT], [1, N]])

        def emit_mains(g):
            xg = xin.tile([128, XW], BF16, tag="xg", name="xg")
            yg = xin.tile([128, XW], BF16, tag="yg", name="yg")
            nc.sync.dma_start(out=xg[:], in_=xg_d[g])
            nc.sync.dma_start(out=yg[:], in_=yg_d[g])
            # phi[d]: [128, 784] = [sample_a(phix|phiy) | sample_b(...)]
            phi = [phip.tile([128, 2 * W2T], BF16, tag=f"phi_{d}",
                             name=f"phi_{d}") for d in range(DT)]
            for d in range(DT):
                psx = ps.tile([128, W2T], F32, tag="psx", name="psx", bufs=3)
                psy = ps.tile([128, W2T], F32, tag="psy", name="psy", bufs=3)
                for k in range(DT):
                    nc.tensor.matmul(psx[:], w1_sb[k][:, d * 128:(d + 1) * 128],
                                     xg[:, k * W2T:(k + 1) * W2T],
                                     start=(k == 0), stop=(k == DT - 1))
                for k in range(DT):
                    nc.tensor.matmul(psy[:], w2_sb[k][:, d * 128:(d + 1) * 128],
                                     yg[:, k * W2T:(k + 1) * W2T],
                                     start=(k == 0), stop=(k == DT - 1))
                # relu evictions: psx = [a|b] of stream x -> phi[d] strided
                outx = strided2(phi[d], 0, W2T, N)
                outy = strided2(phi[d], N, W2T, N)
                nc.scalar.activation(outx, psx[:], ACTF.Relu, bias=b1t[:, d:d + 1])
                nc.scalar.activation(outy, psy[:], ACTF.Relu, bias=b2t[:, d:d + 1])
            return xg, yg, phi

        def emit_sred(g, phi):
            """DVE z-weighted reductions + s' build; overlaps mains(g)."""
            sreps = []
            for i in range(G):
                t_sb = sp.tile([128, DT], BF16, tag=f"t_{i}", name=f"t_{i}")
                s_rep = sp.tile([128, C], BF16, tag=f"srep_{i}",
                                name=f"srep_{i}")
                for d in range(DT):
                    scr = sp.tile([128, W2T], BF16, tag="scr", name="scr")
                    nc.vector.scalar_tensor_tensor(
                        out=scr[:], in0=phi[d][:, i * W2T:(i + 1) * W2T],
                        scalar=ones_bf[:, 0:1], in1=zb[:], op0=ALU.mult,
                        op1=ALU.mult, accum_out=t_sb[:, d:d + 1])
                # s_rep = u3 + t (t broadcast 128-wide per d-block), bf16
                tb = t_sb[:]
                t_bcast = bass.AP(tb.tensor, tb.offset,
                                  [[tb.ap[0][0], 128], [1, DT], [0, 128]])
                sr = s_rep[:]
                sr3 = bass.AP(sr.tensor, sr.offset,
                              [[sr.ap[0][0], 128], [128, DT], [1, 128]])
                u3r3 = u3rep[:].rearrange("p (d c) -> p d c", d=DT, c=128)
                nc.vector.tensor_tensor(sr3, u3r3, t_bcast, ALU.add)
                sreps.append(s_rep)
            return sreps

        def emit_rest(g, xg, yg, phi, sreps, last):
            """psw matvec + W eviction + final reweighting for group g."""
            for i in range(G):
                psw = ps.tile([128, W2T], F32, tag=f"psw_{i}",
                              name=f"psw_{i}", bufs=1)
                for d in range(DT):
                    nc.tensor.matmul(psw[:], sreps[i][:, d * 128:(d + 1) * 128],
                                     phi[d][:, i * W2T:(i + 1) * W2T],
                                     start=(d == 0), stop=(d == DT - 1))
                wxy = sp.tile([128, W2T], BF16, tag=f"wxy_{i}",
                              name=f"wxy_{i}")
                nc.scalar.activation(wxy[:], psw[:], ACTF.Copy, bias=c0)

                # out = x*Wx (DVE) ; y*Wy (Pool) ; add (Pool; DVE when last)
                gx = op.tile([128, ON], BF16, tag=f"gx_{i}", name=f"gx_{i}")
                gy = op.tile([128, ON], BF16, tag=f"gy_{i}", name=f"gy_{i}")
                osb = op.tile([128, ON], BF16, tag=f"osb_{i}", name=f"osb_{i}")
                nc.vector.tensor_tensor(compact_ap(gx), sample_ap(xg, i),
                                        bcast_ap(wxy, 0, N), ALU.mult)
                nc.vector.tensor_tensor(compact_ap(gy), sample_ap(yg, i),
                                        bcast_ap(wxy, N, N), ALU.mult)
                nc.vector.tensor_tensor(osb[:], gx[:], gy[:], ALU.add)
                nc.sync.dma_start(out=out_d[G * g + i], in_=osb[:])

        prev = None
        for g in range(NG):
            cur = emit_mains(g)
            sreps = emit_sred(g, cur[2])
            if prev is not None:
                pg, (pxg, pyg, pphi), psreps = prev
                emit_rest(pg, pxg, pyg, pphi, psreps, False)
            prev = (g, cur, sreps)
        pg, (pxg, pyg, pphi), psreps = prev
        emit_rest(pg, pxg, pyg, pphi, psreps, True)

    if split:
        _split_multi_waits(nc)
    return nc


def _split_multi_waits(nc):
    """This walrus build accepts at most ONE sync-wait per TPB instruction;
    the Tile scheduler emits several. Hoist all but the last wait onto
    same-engine EventSemaphore ops placed immediately before."""
    import json
    data = json.loads(nc.to_json_bytes())
    n = 0
    for fn in data["functions"]:
        for blk in fn["blocks"]:
            out = []
            for inst in blk["instructions"]:
                si = inst.get("sync_info")
                ow = (si or {}).get("on_wait") or []
                if len(ow) > 1:
                    for w in ow[:-1]:
                        n += 1
                        out.append({
                            "name": f"eswait_{n}",
                            "opcode": "EventSemaphore",
                            "engine": inst["engine"],
                            "ins": [],
                            "outs": [],
                            "sync_info": {"on_wait": [w], "on_update": []},
                        })
                    si["on_wait"] = [ow[-1]]
                out.append(inst)
            blk["instructions"] = out
    nc.m = mybir.module_from_json_bytes(json.dumps(data).encode())
    return nc


def prep_host(inputs: dict):
    bf = ml_dtypes.bfloat16
    x = np.ascontiguousarray(np.asarray(inputs["x"], dtype=np.float32))
    y = np.ascontiguousarray(np.asarray(inputs["y"], dtype=np.float32))
    W1 = np.asarray(inputs["W1"], dtype=np.float32)
    W2 = np.asarray(inputs["W2"], dtype=np.float32)
    g1 = np.asarray(inputs["g1"], dtype=np.float32)
    g2 = np.asarray(inputs["g2"], dtype=np.float32)
    b1 = np.asarray(inputs["b1"], dtype=np.float32)
    b2 = np.asarray(inputs["b2"], dtype=np.float32)
    be1 = np.asarray(inputs["be1"], dtype=np.float32)
    be2 = np.asarray(inputs["be2"], dtype=np.float32)
    W3 = np.asarray(inputs["W3"], dtype=np.float32)
    b3 = np.asarray(inputs["b3"], dtype=np.float32)
    W4 = np.asarray(inputs["W4"], dtype=np.float32)
    b4 = np.asarray(inputs["b4"], dtype=np.float32)
    W5 = np.asarray(inputs["W5"], dtype=np.float32)
    b5 = np.asarray(inputs["b5"], dtype=np.float32)

    W1p = np.ascontiguousarray(W1 * g1[None, :]).astype(bf)
    W2p = np.ascontiguousarray(W2 * g2[None, :]).astype(bf)
    b1p = b1 * g1 + be1
    b2p = b2 * g2 + be2
    W5a, W5b = W5[:C, 0], W5[C:, 0]
    u3 = (W3 @ W5a).astype(np.float32)
    u4 = (W4 @ W5b).astype(np.float32)
    z = (u4[:2 * N] + u4[2 * N:]).astype(np.float32)
    c0 = float(b3 @ W5a + b4 @ W5b + b5[0])

    # [B,N,C] -> per-core groups [M, NG, 128, 6*392], blocks [x_a | x_b]
    def pack(a):
        at = a.transpose(0, 2, 1).reshape(NCORES, S, DT, 128, N)
        pair = at.reshape(NCORES, NG, G, DT, 128, N)
        gg = np.concatenate([pair[:, :, 0], pair[:, :, 1]], axis=-1)
        return np.ascontiguousarray(
            gg.transpose(0, 1, 3, 2, 4).reshape(NCORES, NG, 128, XW)).astype(bf)

    XG, YG = pack(x), pack(y)
    zbv = np.ascontiguousarray(np.broadcast_to(z[None, :], (128, W2T))).astype(bf)
    u3t = u3.reshape(DT, 128).T                                 # [128, 6]
    u3r = np.ascontiguousarray(np.repeat(u3t, 128, axis=1)).astype(bf)
    b1t = np.ascontiguousarray(b1p.reshape(DT, 128).T)
    b2t = np.ascontiguousarray(b2p.reshape(DT, 128).T)

    in_maps = []
    for cidx in range(NCORES):
        in_maps.append({
            "xg": XG[cidx], "yg": YG[cidx], "w1": W1p, "w2": W2p,
            "zb": zbv, "u3r": u3r, "b1": b1t, "b2": b2t,
        })
    return in_maps, c0, x, y


def unpack_out(results) -> np.ndarray:
    outs = []
    for cidx in range(NCORES):
        o = np.asarray(results[cidx]["out"]).astype(np.float32)  # [S,128,ON]
        o = o.reshape(S, 128, DT, N).transpose(0, 2, 1, 3).reshape(S, C, N)
        outs.append(o.transpose(0, 2, 1))     # [S, N, C]
    return np.ascontiguousarray(np.concatenate(outs, axis=0))


def kernel(**inputs) -> np.ndarray:
    in_maps, c0, _, _ = prep_host(inputs)
    nc = build_bass(c0)
    res = run_bass_kernel_spmd(nc, in_maps, list(range(NCORES)))
    return unpack_out(res.results)


# revision 29
# speedup vs baseline: 1.1968x; 1.1484x over previous
"""Trainium2 Bass kernel for the CRA relation module (bf16 v2).

Math (per sample, derived from symmetry of A = cat_phi cat_phi^T):
    phi_x = relu(x@W1p + b1p), phi_y = relu(y@W2p + b2p)
    s  = u3 + phi_x^T z[:196] + phi_y^T z[196:]      (768-vector)
    W  = [phi_x|phi_y] @ s + c0                       (392 scalars)
    out = x * W[:196] + y * W[196:]

Device schedule per group of G=2 samples (8 groups/core, 16 samples/core,
pure data-parallel over 8 cores):
  PE   : 72 bf16 matmuls (768x768 "1x1 conv", both streams), then per
         sample 6 matvec matmuls with lhsT = s' replicated to 128 cols
         -> psw[128,392] (W on every partition)
  ACT  : 12 relu evictions PSUM->SBUF bf16 (strided out AP fills both
         samples' phi slots), 2 wxy evictions (+c0 -> bf16)
  DVE  : 12 z-weighted reductions (scalar_tensor_tensor accum), s'
         replication (tensor_scalar with per-partition t), x*Wx mult,
         final add
  Pool : y*Wy mult (tensor_tensor, broadcast-d AP on Wy)
  DMA  : xg/yg in (bf16), out (bf16; host casts fp32)

Layouts: feature-major. xg[group] = [128, 6*392] bf16, each 392 block =
[x_a(196) | x_b(196)] for one cin tile. Broadcast of W along the d-repeat
uses 0-stride APs (verified supported by walrus + sim).
"""

import numpy as np
import ml_dtypes
from contextlib import ExitStack

import concourse.bass as bass
import concourse.tile as tile
import concourse.mybir as mybir
from concourse.bass_utils import run_bass_kernel_spmd

F32 = mybir.dt.float32
BF16 = mybir.dt.bfloat16
F8 = mybir.dt.float8e4
ALU = mybir.AluOpType
ACTF = mybir.ActivationFunctionType
DR = mybir.MatmulPerfMode.DoubleRow

B, N, C = 128, 196, 768
NCORES = 8
S = B // NCORES          # 16 samples per core
G = 2                    # samples per group
NG = S // G              # 8 groups per core
DT = C // 128            # 6 feature tiles
KK = DT // 2             # 3 DoubleRow k-pair steps
W2T = 2 * N              # 392
PAIR = 400               # fp8 per-k block incl. 8 pad cols (step%16==0)
XW = DT * W2T            # 2352 cols per xg tile
X8W = KK * 2 * PAIR      # 2400 cols per fp8 xg tile
ON = DT * N              # 1176 out cols per sample


def build_bass(c0: float, split: bool = True) -> bass.Bass:
    nc = bass.Bass()
    xg_d = nc.declare_dram_parameter("xg", [NG, 128, XW], BF16, isOutput=False)
    xg8_d = nc.declare_dram_parameter("xg8", [NG, 128, X8W], F8, isOutput=False)
    yg_d = nc.declare_dram_parameter("yg", [NG, 128, XW], BF16, isOutput=False)
    w18_d = nc.declare_dram_parameter("w18", [KK, 128, 2 * C], F8, isOutput=False)
    w2_d = nc.declare_dram_parameter("w2", [C, C], BF16, isOutput=False)
    zb_d = nc.declare_dram_parameter("zb", [128, W2T], BF16, isOutput=False)
    u3r_d = nc.declare_dram_parameter("u3r", [128, C], BF16, isOutput=False)
    b1_d = nc.declare_dram_parameter("b1", [128, DT], F32, isOutput=False)
    b2_d = nc.declare_dram_parameter("b2", [128, DT], F32, isOutput=False)
    out_d = nc.declare_dram_parameter("out", [S, 128, ON], BF16, isOutput=True)

    with tile.TileContext(nc) as tc, ExitStack() as ctx:
        const = ctx.enter_context(tc.tile_pool(name="const", bufs=1))

        # Weights + consts go on the ACT DMA queue so the first groups'
        # xg/yg (SP queue) land in parallel; k=0 weight tiles first.
        w18_sb, w2_sb = [], []
        for kk in range(KK):
            t1 = const.tile([128, 2 * C], F8, tag=f"w18_{kk}")
            nc.scalar.dma_start(out=t1[:], in_=w18_d[kk])
            w18_sb.append(t1)
            for k in (2 * kk, 2 * kk + 1):
                t2 = const.tile([128, C], BF16, tag=f"w2_{k}")
                nc.scalar.dma_start(out=t2[:], in_=w2_d[k * 128:(k + 1) * 128, :])
                w2_sb.append(t2)

        zb = const.tile([128, W2T], BF16, tag="zb")
        nc.scalar.dma_start(out=zb[:], in_=zb_d[:, :])
        u3rep = const.tile([128, C], BF16, tag="u3r")
        nc.scalar.dma_start(out=u3rep[:], in_=u3r_d[:, :])
        b1t = const.tile([128, DT], F32, tag="b1")
        nc.scalar.dma_start(out=b1t[:], in_=b1_d[:, :])
        b2t = const.tile([128, DT], F32, tag="b2")
        nc.scalar.dma_start(out=b2t[:], in_=b2_d[:, :])
        ones_bf = const.tile([128, 2], BF16, tag="ones_bf")
        nc.vector.memset(ones_bf[:], 1.0)
        # Absorb bias-tile DMA deps into ACT program order once (ISA
        # Activation descriptor holds a single sync-wait).
        warm1 = const.tile([128, 1], F32, tag="warm1")
        warm2 = const.tile([128, 1], F32, tag="warm2")
        nc.scalar.activation(warm1[:], b1t[:, 0:1], ACTF.Copy)
        nc.scalar.activation(warm2[:], b2t[:, 0:1], ACTF.Copy)

        xin = ctx.enter_context(tc.tile_pool(name="xin", bufs=3))
        phip = ctx.enter_context(tc.tile_pool(name="phi", bufs=3))
        sp = ctx.enter_context(tc.tile_pool(name="sp", bufs=3))
        op = ctx.enter_context(tc.tile_pool(name="op", bufs=2))
        ps = ctx.enter_context(tc.tile_pool(name="ps", bufs=2, space="PSUM"))

        def strided2(tile_, offset, step, n, parts=128):
            """AP [parts, 2 (col step), n] at col offset within the tile."""
            t = tile_[:]
            return bass.AP(t.tensor, t.offset + offset,
                           [[t.ap[0][0], parts], [step, 2], [1, n]])

        def sample_ap(tile_, i):
            """AP [128, DT (step W2T), N]: sample i's stream cols of xg/yg."""
            t = tile_[:]
            return bass.AP(t.tensor, t.offset + i * N,
                           [[t.ap[0][0], 128], [W2T, DT], [1, N]])

        def bcast_ap(tile_, offset, n):
            """AP [128, DT (stride 0), n]: wxy cols broadcast over d."""
            t = tile_[:]
            return bass.AP(t.tensor, t.offset + offset,
                           [[t.ap[0][0], 128], [0, DT], [1, n]])

        def compact_ap(tile_, parts=128):
            t = tile_[:]
            return bass.AP(t.tensor, t.offset,
                           [[t.ap[0][0], parts], [N, DT], [1, N]])

        def emit_mains(g):
            xg = xin.tile([128, XW], BF16, tag="xg", name="xg")
            xg8 = xin.tile([128, X8W], F8, tag="xg8", name="xg8")
            yg = xin.tile([128, XW], BF16, tag="yg", name="yg")
            nc.sync.dma_start(out=xg8[:], in_=xg8_d[g])
            nc.sync.dma_start(out=yg[:], in_=yg_d[g])
            nc.sync.dma_start(out=xg[:], in_=xg_d[g])
            # phi[d]: [128, 784] = [sample_a(phix|phiy) | sample_b(...)]
            phi = [phip.tile([128, 2 * W2T], BF16, tag=f"phi_{d}",
                             name=f"phi_{d}") for d in range(DT)]
            x8t = xg8[:]
            for d in range(DT):
                psx = ps.tile([128, W2T], F32, tag="psx", name="psx", bufs=3)
                psy = ps.tile([128, W2T], F32, tag="psy", name="psy", bufs=3)
                for kk in range(KK):
                    wt = w18_sb[kk][:]
                    lhsT = bass.AP(wt.tensor, wt.offset + d * 128,
                                   [[wt.ap[0][0], 128], [C, 2], [1, 128]])
                    rhs = bass.AP(x8t.tensor, x8t.offset + kk * 2 * PAIR,
                                  [[x8t.ap[0][0], 128], [PAIR, 2], [1, W2T]])
                    nc.tensor.matmul(psx[:], lhsT, rhs, perf_mode=DR,
                                     start=(kk == 0), stop=(kk == KK - 1))
                for k in range(DT):
                    nc.tensor.matmul(psy[:], w2_sb[k][:, d * 128:(d + 1) * 128],
                                     yg[:, k * W2T:(k + 1) * W2T],
                                     start=(k == 0), stop=(k == DT - 1))
                # relu evictions: psx = [a|b] of stream x -> phi[d] strided
                outx = strided2(phi[d], 0, W2T, N)
                outy = strided2(phi[d], N, W2T, N)
                nc.scalar.activation(outx, psx[:], ACTF.Relu, bias=b1t[:, d:d + 1])
                nc.scalar.activation(outy, psy[:], ACTF.Relu, bias=b2t[:, d:d + 1])
            return xg, yg, phi

        def emit_sred(g, phi):
            """DVE z-weighted reductions + s' build; overlaps mains(g)."""
            sreps = []
            for i in range(G):
                t_sb = sp.tile([128, DT], BF16, tag=f"t_{i}", name=f"t_{i}")
                s_rep = sp.tile([128, C], BF16, tag=f"srep_{i}",
                                name=f"srep_{i}")
                for d in range(DT):
                    scr = sp.tile([128, W2T], BF16, tag="scr", name="scr")
                    nc.vector.scalar_tensor_tensor(
                        out=scr[:], in0=phi[d][:, i * W2T:(i + 1) * W2T],
                        scalar=ones_bf[:, 0:1], in1=zb[:], op0=ALU.mult,
                        op1=ALU.mult, accum_out=t_sb[:, d:d + 1])
                # s_rep = u3 + t (t broadcast 128-wide per d-block), bf16
                tb = t_sb[:]
                t_bcast = bass.AP(tb.tensor, tb.offset,
                                  [[tb.ap[0][0], 128], [1, DT], [0, 128]])
                sr = s_rep[:]
                sr3 = bass.AP(sr.tensor, sr.offset,
                              [[sr.ap[0][0], 128], [128, DT], [1, 128]])
                u3r3 = u3rep[:].rearrange("p (d c) -> p d c", d=DT, c=128)
                nc.vector.tensor_tensor(sr3, u3r3, t_bcast, ALU.add)
                sreps.append(s_rep)
            return sreps

        def emit_rest(g, xg, yg, phi, sreps, last):
            """psw matvec + W eviction + final reweighting for group g."""
            for i in range(G):
                psw = ps.tile([128, W2T], F32, tag=f"psw_{i}",
                              name=f"psw_{i}", bufs=1)
                for d in range(DT):
                    nc.tensor.matmul(psw[:], sreps[i][:, d * 128:(d + 1) * 128],
                                     phi[d][:, i * W2T:(i + 1) * W2T],
                                     start=(d == 0), stop=(d == DT - 1))
                wxy = sp.tile([128, W2T], BF16, tag=f"wxy_{i}",
                              name=f"wxy_{i}")
                nc.scalar.activation(wxy[:], psw[:], ACTF.Copy, bias=c0)

                # out = x*Wx (DVE) ; y*Wy (Pool) ; add (Pool; DVE when last)
                gx = op.tile([128, ON], BF16, tag=f"gx_{i}", name=f"gx_{i}")
                gy = op.tile([128, ON], BF16, tag=f"gy_{i}", name=f"gy_{i}")
                osb = op.tile([128, ON], BF16, tag=f"osb_{i}", name=f"osb_{i}")
                nc.vector.tensor_tensor(compact_ap(gx), sample_ap(xg, i),
                                        bcast_ap(wxy, 0, N), ALU.mult)
                nc.vector.tensor_tensor(compact_ap(gy), sample_ap(yg, i),
                                        bcast_ap(wxy, N, N), ALU.mult)
                nc.vector.tensor_tensor(osb[:], gx[:], gy[:], ALU.add)
                nc.sync.dma_start(out=out_d[G * g + i], in_=osb[:])

        prev = None
        for g in range(NG):
            cur = emit_mains(g)
            sreps = emit_sred(g, cur[2])
            if prev is not None:
                pg, (pxg, pyg, pphi), psreps = prev
                emit_rest(pg, pxg, pyg, pphi, psreps, False)
            prev = (g, cur, sreps)
        pg, (pxg, pyg, pphi), psreps = prev
        emit_rest(pg, pxg, pyg, pphi, psreps, True)

    if split:
        _split_multi_waits(nc)
    return nc


def _split_multi_waits(nc):
    """This walrus build accepts at most ONE sync-wait per TPB instruction;
    the Tile scheduler emits several. Hoist all but the last wait onto
    same-engine EventSemaphore ops placed immediately before."""
    import json
    data = json.loads(nc.to_json_bytes())
    n = 0
    for fn in data["functions"]:
        for blk in fn["blocks"]:
            out = []
            for inst in blk["instructions"]:
                si = inst.get("sync_info")
                ow = (si or {}).get("on_wait") or []
                if len(ow) > 1:
                    for w in ow[:-1]:
                        n += 1
                        out.append({
                            "name": f"eswait_{n}",
                            "opcode": "EventSemaphore",
                            "engine": inst["engine"],
                            "ins": [],
                            "outs": [],
                            "sync_info": {"on_wait": [w], "on_update": []},
                        })
                    si["on_wait"] = [ow[-1]]
                out.append(inst)
            blk["instructions"] = out
    nc.m = mybir.module_from_json_bytes(json.dumps(data).encode())
    return nc


def prep_host(inputs: dict):
    bf = ml_dtypes.bfloat16
    x = np.ascontiguousarray(np.asarray(inputs["x"], dtype=np.float32))
    y = np.ascontiguousarray(np.asarray(inputs["y"], dtype=np.float32))
    W1 = np.asarray(inputs["W1"], dtype=np.float32)
    W2 = np.asarray(inputs["W2"], dtype=np.float32)
    g1 = np.asarray(inputs["g1"], dtype=np.float32)
    g2 = np.asarray(inputs["g2"], dtype=np.float32)
    b1 = np.asarray(inputs["b1"], dtype=np.float32)
    b2 = np.asarray(inputs["b2"], dtype=np.float32)
    be1 = np.asarray(inputs["be1"], dtype=np.float32)
    be2 = np.asarray(inputs["be2"], dtype=np.float32)
    W3 = np.asarray(inputs["W3"], dtype=np.float32)
    b3 = np.asarray(inputs["b3"], dtype=np.float32)
    W4 = np.asarray(inputs["W4"], dtype=np.float32)
    b4 = np.asarray(inputs["b4"], dtype=np.float32)
    W5 = np.asarray(inputs["W5"], dtype=np.float32)
    b5 = np.asarray(inputs["b5"], dtype=np.float32)

    f8 = ml_dtypes.float8_e4m3
    W1p = np.ascontiguousarray(W1 * g1[None, :])
    W2p = np.ascontiguousarray(W2 * g2[None, :]).astype(bf)
    # w18[kk, p, j, c] = W1p[(2kk+j)*128 + p, c], fp8
    W18 = np.ascontiguousarray(
        W1p.reshape(KK, 2, 128, C).transpose(0, 2, 1, 3)
        .reshape(KK, 128, 2 * C)).astype(f8)
    b1p = b1 * g1 + be1
    b2p = b2 * g2 + be2
    W5a, W5b = W5[:C, 0], W5[C:, 0]
    u3 = (W3 @ W5a).astype(np.float32)
    u4 = (W4 @ W5b).astype(np.float32)
    z = (u4[:2 * N] + u4[2 * N:]).astype(np.float32)
    c0 = float(b3 @ W5a + b4 @ W5b + b5[0])

    # [B,N,C] -> per-core groups [M, NG, 128, 6*392], blocks [x_a | x_b]
    def pack(a):
        at = a.transpose(0, 2, 1).reshape(NCORES, S, DT, 128, N)
        pair = at.reshape(NCORES, NG, G, DT, 128, N)
        gg = np.concatenate([pair[:, :, 0], pair[:, :, 1]], axis=-1)
        return np.ascontiguousarray(
            gg.transpose(0, 1, 3, 2, 4).reshape(NCORES, NG, 128, XW)).astype(bf)

    XG, YG = pack(x), pack(y)

    # fp8 xg with k-pair blocks: [.. , kk, (j, 400-block)] where each
    # 400-block = [x_{k=2kk+j} (392) | 0*8]
    def pack8(a):
        at = a.transpose(0, 2, 1).reshape(NCORES, S, DT, 128, N)
        pair = at.reshape(NCORES, NG, G, DT, 128, N)
        gg = np.concatenate([pair[:, :, 0], pair[:, :, 1]], axis=-1)
        blk = np.zeros((NCORES, NG, DT, 128, PAIR), dtype=np.float32)
        blk[..., 0:W2T] = gg                          # [M,NG,DT,128,400]
        return np.ascontiguousarray(
            blk.transpose(0, 1, 3, 2, 4).reshape(NCORES, NG, 128, X8W)).astype(f8)

    XG8 = pack8(x)
    zbv = np.ascontiguousarray(np.broadcast_to(z[None, :], (128, W2T))).astype(bf)
    u3t = u3.reshape(DT, 128).T                                 # [128, 6]
    u3r = np.ascontiguousarray(np.repeat(u3t, 128, axis=1)).astype(bf)
    b1t = np.ascontiguousarray(b1p.reshape(DT, 128).T)
    b2t = np.ascontiguousarray(b2p.reshape(DT, 128).T)

    in_maps = []
    for cidx in range(NCORES):
        in_maps.append({
            "xg": XG[cidx], "xg8": XG8[cidx], "yg": YG[cidx],
            "w18": W18, "w2": W2p,
            "zb": zbv, "u3r": u3r, "b1": b1t, "b2": b2t,
        })
    return in_maps, c0, x, y


def unpack_out(results) -> np.ndarray:
    outs = []
    for cidx in range(NCORES):
        o = np.asarray(results[cidx]["out"]).astype(np.float32)  # [S,128,ON]
        o = o.reshape(S, 128, DT, N).transpose(0, 2, 1, 3).reshape(S, C, N)
        outs.append(o.transpose(0, 2, 1))     # [S, N, C]
    return np.ascontiguousarray(np.concatenate(outs, axis=0))


def kernel(**inputs) -> np.ndarray:
    in_maps, c0, _, _ = prep_host(inputs)
    nc = build_bass(c0)
    res = run_bass_kernel_spmd(nc, in_maps, list(range(NCORES)))
    return unpack_out(res.results)
